# revision 8
# baseline (speedup 1.0000x reference)
"""Trainium2 Bass kernel for BinaryLinearWscales (v3).

Math:  out = x @ (wscale * sign(weight) + wbias).T = x @ w'.T
with w' folded on-chip into bf16: w' = (w>=0)*(2*wscale) + (wbias-wscale).

Only the SIGN of each weight matters, so the host passes the top BYTE of
each f32 weight (sign + exp[7:1]): wsb = weight.view(u8)[:, 3::4], 4 MB
per core instead of 16 MB f32 — the weight DMA head shrinks 4x and runs
on the HWDGE (sync) ring, parallel to x's SWDGE (gpsimd) ring.  On-chip:
(u8 < 128) <=> w >= 0, folded to w' by two DVE tensor_scalar ops with
per-partition [P,1] scale operands.

Sharding: 8 cores as 2 token-groups x 4 feature-groups: 2048 tokens of
x (32 MB f32 read, cast in flight) and 1024 output rows per core.

Schedule (the PE executes in emission order, so production is woven):
  - all 32 weight-byte DMAs emitted up front (HWDGE ring)
  - x strips stream on the SWDGE ring (f32->bf16/f32r cast in flight)
  - prestage: 8 quad-groups of x PE-transposes for tg0
  - tg0's matmul loop weaves in, per k-strip ks: DVE fold -> PE weight
    transposes [128n,128k]->psum->copyback to the resident swT (8 MB);
    matmuls for kc consume swT slices as they land
  - tgs 1..7 stream pure matmuls + just-in-time x transposes
  - epilogue per 128-token block: single PSUM->SBUF copy (ACT/DVE
    alternating) -> HWDGE DMA out

PSUM: pox 3x[128,1024]f32 (6 banks) + shared 1-bank "stage" ring x2
(x-transpose quads and weight-transpose tiles) = 8 banks exactly.
Modes: 'bf16' (default, ~2.5e-3 rel err), 'mixed' (f32r x + exact +-1
sign matrix + xsum matmuls + scale/bias epilogue, ~1e-4 rel err),
PSUM in mixed: pox 2x2 + stage 2 + pss 2x1 = 8.
"""

import os
from contextlib import ExitStack

import numpy as np

P = 128

# full problem dims
B, S, DIN, DOUT = 2, 2048, 4096, 4096
N_CORES = 8
TGROUPS = 2   # token groups
NGROUPS = 4   # feature groups
T_SHARD = B * S // TGROUPS   # 2048 tokens per core
N_SHARD = DOUT // NGROUPS    # 1024 out-features per core


def build_body(ctx, tc, out_ap, x_ap, wsb_ap, wscale_ap, wbias_ap, mode="bf16"):
    import concourse.bass as bass
    from concourse import mybir
    from concourse.bass import ts
    from concourse.masks import make_identity

    nc = tc.nc
    T, K = x_ap.shape          # 2048, 4096
    N, K2 = wsb_ap.shape       # 1024, 4096
    assert K == K2
    assert T % 256 == 0 and K % P == 0 and N % P == 0
    KC = K // P                # 32 k chunks
    NB = N // P                # 8 weight row blocks
    KSTRIP = 1024              # k columns per weight fold/transpose strip
    NKS = K // KSTRIP          # 4 k strips
    KC_PER_KS = KSTRIP // P    # 8
    TGRP = 256
    TB = TGRP // P             # 2
    NTG = T // TGRP            # 8

    f32 = mybir.dt.float32
    bf16 = mybir.dt.bfloat16
    f32r = mybir.dt.float32r
    u8 = mybir.dt.uint8
    Alu = mybir.AluOpType

    fold = mode != "mixed"
    x_dt = f32r if mode == "mixed" else bf16
    # k-chunks packed per 1-bank (2KB) psum staging tile
    CPK = 4 if x_dt == bf16 else 2
    QPT = KC // CPK            # stage groups per token group

    # ---------------- pools ----------------
    XNAT_BUFS = 6 if fold else 4   # x strips of DMA lookahead
    PRESTAGE = 8
    xnat_pool = ctx.enter_context(tc.tile_pool(name="xnat", bufs=XNAT_BUFS))
    xsplit_pool = ctx.enter_context(
        tc.tile_pool(name="xsplit", bufs=PRESTAGE + 2)
    )
    opool = ctx.enter_context(tc.tile_pool(name="opool", bufs=3))
    wu_pool = ctx.enter_context(tc.tile_pool(name="wu", bufs=NKS * NB))
    ws_pool = ctx.enter_context(tc.tile_pool(name="ws", bufs=NB + 2))

    def load_x_group(tg):
        """DMA the tg-th 256-token block of x (2 x 128-row strips),
        casting f32 -> bf16/f32r in flight (SWDGE)."""
        x_nats = []
        for tb in range(TB):
            x_nat = xnat_pool.tile(
                [P, K], x_dt, name=f"x_nat_{tg}_{tb}", tag="x_nat",
                bufs=XNAT_BUFS,
            )
            row = ts(tg * TB + tb, P)
            pieces = 4 if (tg == 0 and tb == 0) else 2
            step = K // pieces
            for pc in range(pieces):
                nc.gpsimd.dma_start(
                    x_nat[:, pc * step:(pc + 1) * step],
                    x_ap[row, pc * step:(pc + 1) * step],
                )
            x_nats.append(x_nat)
        return x_nats

    # prefetch the first x group before anything else so the PE starts early
    x_groups = {0: load_x_group(0)}

    # ---------------- weight-byte DMAs, all up front (HWDGE ring) -------
    wu = {}
    for ks in range(NKS):
        col = slice(ks * KSTRIP, (ks + 1) * KSTRIP)
        for nb in range(NB):
            t = wu_pool.tile(
                [P, KSTRIP], u8, name=f"wu_{ks}_{nb}", tag="wu",
                bufs=NKS * NB,
            )
            nc.sync.dma_start(t[:], wsb_ap[ts(nb, P), col])
            wu[ks, nb] = t

    # ---------------- constants ----------------
    const = ctx.enter_context(tc.tile_pool(name="const", bufs=1))
    ident_f32 = const.tile([P, P], f32, name="ident_f32", tag="ident_f32")
    make_identity(nc, ident_f32)
    ident_x = const.tile([P, P], x_dt, name="ident_x", tag="ident_x")
    nc.vector.tensor_copy(ident_x[:], ident_f32[:])
    if x_dt == bf16:
        ident_bf16 = ident_x
    else:
        ident_bf16 = const.tile([P, P], bf16, name="ident_b", tag="ident_b")
        nc.vector.tensor_copy(ident_bf16[:], ident_f32[:])

    # per-output-row scale columns: wcol[p, nb] = val[nb*128 + p]
    wsc_col = const.tile([P, NB], f32, name="wsc_col", tag="wsc_col")
    nc.sync.dma_start(wsc_col[:], wscale_ap[:, 0].rearrange("(i p) -> p i", p=P))
    wbi_col = const.tile([P, NB], f32, name="wbi_col", tag="wbi_col")
    nc.sync.dma_start(wbi_col[:], wbias_ap[:, 0].rearrange("(i p) -> p i", p=P))
    if fold:
        # w' = (sign>=0) * ws2 + wbm,  ws2 = 2*wscale, wbm = wbias - wscale
        ws2_col = const.tile([P, NB], f32, name="ws2_col", tag="ws2_col")
        nc.vector.tensor_scalar(
            out=ws2_col[:], in0=wsc_col[:], scalar1=2.0, scalar2=None,
            op0=Alu.mult,
        )
        wbm_col = const.tile([P, NB], f32, name="wbm_col", tag="wbm_col")
        nc.vector.tensor_sub(wbm_col[:], wbi_col[:], wsc_col[:])
    else:
        ONESW = 8
        ones_stage = const.tile([P, ONESW], f32, name="ones_st", tag="ones_st")
        nc.vector.memset(ones_stage[:], 1.0)
        ones_col = const.tile([P, ONESW], bf16, name="ones_col", tag="ones_col")
        nc.vector.tensor_copy(ones_col[:], ones_stage[:])
        wsc_stage = const.tile([1, N], f32, name="wsc_stage", tag="wsc_stage")
        nc.sync.dma_start(wsc_stage[:], wscale_ap[:, 0].rearrange("n -> 1 n"))
        wbi_stage = const.tile([1, N], f32, name="wbi_stage", tag="wbi_stage")
        nc.sync.dma_start(wbi_stage[:], wbias_ap[:, 0].rearrange("n -> 1 n"))
        wscale_rep = const.tile([P, N], f32, name="wscale_rep", tag="wscale_rep")
        nc.gpsimd.partition_broadcast(wscale_rep[:], wsc_stage[:])
        wbias_rep = const.tile([P, N], f32, name="wbias_rep", tag="wbias_rep")
        nc.gpsimd.partition_broadcast(wbias_rep[:], wbi_stage[:])

    def get_x(tg):
        if tg not in x_groups:
            x_groups[tg] = load_x_group(tg)
        return x_groups[tg]

    # shared 1-bank psum staging ring (x quads + weight transpose tiles)
    stage_pool = ctx.enter_context(
        tc.tile_pool(name="stage", bufs=2, space="PSUM")
    )
    _cb_flip = [0]

    def copyback(dst, src):
        # alternate ACT / DVE to balance engine load
        if _cb_flip[0] % 2 == 0:
            nc.scalar.copy(dst, src)
        else:
            nc.vector.tensor_copy(dst, src)
        _cb_flip[0] += 1

    def stage_group(q):
        """PE-transpose stage group q (CPK k-chunks of one token group) and
        copy it back to SBUF in one op."""
        tg, j = divmod(q, QPT)
        x_nats = get_x(tg)
        psx = stage_pool.tile(
            [P, CPK * TGRP], x_dt, name=f"psx_{q}", tag="stage"
        )
        for dk in range(CPK):
            kc = j * CPK + dk
            for tb in range(TB):
                nc.tensor.transpose(
                    psx[:, dk * TGRP + tb * P: dk * TGRP + (tb + 1) * P],
                    x_nats[tb][:, ts(kc, P)],
                    ident_x,
                )
        xq = xsplit_pool.tile(
            [P, CPK * TGRP], x_dt, name=f"xq_{q}", tag="xq",
            bufs=PRESTAGE + 2,
        )
        copyback(xq[:], psx[:])
        return xq

    # ---------------- weight fold + transpose (woven into tg0) ----------
    swt_pool = ctx.enter_context(tc.tile_pool(name="swt", bufs=1))
    sT_all = swt_pool.tile([P, KC * N], bf16, name="sT_all", tag="sT_all")

    def sT(kc):
        return sT_all[:, kc * N:(kc + 1) * N]

    def process_ks(ks):
        """DVE-fold one k strip of weight bytes and PE-transpose it into
        sT_all (via the shared psum stage ring)."""
        s_nats = []
        for nb in range(NB):
            s = ws_pool.tile(
                [P, KSTRIP], bf16, name=f"ws_{ks}_{nb}", tag="ws",
                bufs=NB + 2,
            )
            if fold:
                nc.vector.tensor_scalar(
                    out=s[:], in0=wu[ks, nb][:],
                    scalar1=128, scalar2=ws2_col[:, nb:nb + 1],
                    op0=Alu.is_lt, op1=Alu.mult,
                )
                nc.vector.tensor_scalar(
                    out=s[:], in0=s[:],
                    scalar1=wbm_col[:, nb:nb + 1], scalar2=None,
                    op0=Alu.add,
                )
            else:
                nc.vector.tensor_scalar(
                    out=s[:], in0=wu[ks, nb][:],
                    scalar1=128, scalar2=2.0, op0=Alu.is_lt, op1=Alu.mult,
                )
                nc.vector.tensor_scalar(
                    out=s[:], in0=s[:],
                    scalar1=1.0, scalar2=None, op0=Alu.subtract,
                )
            s_nats.append(s)
        for dkc in range(KC_PER_KS):
            kc = ks * KC_PER_KS + dkc
            psw = stage_pool.tile([P, N], bf16, name=f"psw_{kc}", tag="stage")
            for nb in range(NB):
                nc.tensor.transpose(
                    psw[:, nb * P:(nb + 1) * P],
                    s_nats[nb][:, ts(dkc, P)],
                    ident_bf16,
                )
            copyback(sT(kc), psw[:])

    # prestage tg0's x transposes, then the first weight strip
    NQ = NTG * QPT
    staged = [stage_group(q) for q in range(min(PRESTAGE, NQ))]
    process_ks(0)

    # weight strip s is needed by tg0 iteration j = s*QPT//NKS; emit one
    # iteration earlier so the PE pipeline never waits on emission order
    weave = {(s * QPT // NKS) - 1: s for s in range(1, NKS)}

    # ---------------- main phase ----------------
    pox_pool = ctx.enter_context(
        tc.tile_pool(name="pox", bufs=3 if fold else 2, space="PSUM")
    )
    if not fold:
        pss_pool = ctx.enter_context(
            tc.tile_pool(name="pss", bufs=2, space="PSUM")
        )

    for tg in range(NTG):
        psum_os = [
            pox_pool.tile([P, N], f32, name=f"po_{tg}_{ot}", tag="po")
            for ot in range(TB)
        ]
        if not fold:
            psum_ss = [
                pss_pool.tile([P, ONESW], f32, name=f"ps_{tg}_{ot}", tag="ps")
                for ot in range(TB)
            ]
        for j in range(QPT):
            xq = staged.pop(0)
            q = tg * QPT + j
            if q + PRESTAGE < NQ:
                staged.append(stage_group(q + PRESTAGE))
            if tg == 0 and j in weave:
                s = weave[j]
                process_ks(s)
                get_x(s + 1)
            for dk in range(CPK):
                kc = j * CPK + dk
                for ot in range(TB):
                    lhs = xq[:, dk * TGRP + ot * P: dk * TGRP + (ot + 1) * P]
                    # one matmul may write at most 512 f32 psum columns
                    for h in range(N // 512):
                        nc.tensor.matmul(
                            psum_os[ot][:, h * 512:(h + 1) * 512],
                            lhs,
                            sT(kc)[:, h * 512:(h + 1) * 512],
                            start=(kc == 0),
                            stop=(kc == KC - 1),
                        )
                    if not fold:
                        nc.tensor.matmul(
                            psum_ss[ot][:, 0:ONESW],
                            lhs,
                            ones_col[:],
                            start=(kc == 0),
                            stop=(kc == KC - 1),
                        )

        for ot in range(TB):
            out_sb = opool.tile(
                [P, N], f32, name=f"out_sb_{tg}_{ot}", tag="out_sb"
            )
            if fold:
                copyback(out_sb[:], psum_os[ot][:])
            else:
                nc.vector.tensor_mul(out_sb[:], psum_os[ot][:], wscale_rep[:])
                nc.vector.scalar_tensor_tensor(
                    out=out_sb[:],
                    in0=wbias_rep[:],
                    scalar=psum_ss[ot][:, 0:1],
                    in1=out_sb[:],
                    op0=Alu.mult,
                    op1=Alu.add,
                )
            nc.sync.dma_start(out_ap[ts(tg * TB + ot, P), :], out_sb[:])


def build_nc(T, K, N, mode="bf16"):
    import concourse.tile as tile
    from concourse import bacc, mybir

    nc = bacc.Bacc(
        "TRN2",
        target_bir_lowering=False,
        debug=False,
        enable_asserts=False,
    )
    f32 = mybir.dt.float32
    x_t = nc.dram_tensor("x", [T, K], f32, kind="ExternalInput")
    wsb_t = nc.dram_tensor("wsb", [N, K], mybir.dt.uint8, kind="ExternalInput")
    wsc_t = nc.dram_tensor("wscale", [N, 1], f32, kind="ExternalInput")
    wbi_t = nc.dram_tensor("wbias", [N, 1], f32, kind="ExternalInput")
    out_t = nc.dram_tensor("out", [T, N], f32, kind="ExternalOutput")

    with tile.TileContext(nc) as tc:
        with ExitStack() as ctx:
            build_body(
                ctx, tc, out_t.ap(), x_t.ap(), wsb_t.ap(), wsc_t.ap(),
                wbi_t.ap(), mode=mode,
            )
    nc.compile()
    return nc


_NC_CACHE = {}
_LAST_RESULT = None


def _get_nc(T, K, N, mode):
    key = (T, K, N, mode)
    if key not in _NC_CACHE:
        _NC_CACHE[key] = build_nc(T, K, N, mode)
    return _NC_CACHE[key]


def _make_in_maps(inputs):
    x = np.asarray(inputs["x"], dtype=np.float32).reshape(B * S, DIN)
    weight = np.asarray(inputs["weight"], dtype=np.float32)
    # top byte of each little-endian f32: sign + exp[7:1]; sign(w) >= 0
    # on-chip test is (byte < 128)
    wsb_full = weight.view(np.uint8)[:, 3::4]
    wscale = np.asarray(inputs["wscale"], dtype=np.float32).reshape(-1)
    wbias = np.asarray(inputs["wbias"], dtype=np.float32).reshape(-1)
    in_maps = []
    for c in range(N_CORES):
        tg, ng = divmod(c, NGROUPS)
        trows = slice(tg * T_SHARD, (tg + 1) * T_SHARD)
        nrows = slice(ng * N_SHARD, (ng + 1) * N_SHARD)
        in_maps.append(
            {
                "x": np.ascontiguousarray(x[trows]),
                "wsb": np.ascontiguousarray(wsb_full[nrows]),
                "wscale": np.ascontiguousarray(wscale[nrows]).reshape(N_SHARD, 1),
                "wbias": np.ascontiguousarray(wbias[nrows]).reshape(N_SHARD, 1),
            }
        )
    return in_maps


def kernel(x, weight, wscale, wbias):
    from concourse.bass_utils import run_bass_kernel_spmd

    mode = os.environ.get("KERNEL_MODE", "bf16")
    nc = _get_nc(T_SHARD, DIN, N_SHARD, mode)
    in_maps = _make_in_maps(
        {"x": x, "weight": weight, "wscale": wscale, "wbias": wbias}
    )

    trace = os.environ.get("KERNEL_TRACE", "0") == "1"
    res = run_bass_kernel_spmd(
        nc, in_maps, core_ids=list(range(N_CORES)), trace=trace
    )
    global _LAST_RESULT
    _LAST_RESULT = res
    if trace and res.exec_time_ns is not None:
        print(f"HW exec time: {res.exec_time_ns} ns")
    full = np.empty((B * S, DOUT), dtype=np.float32)
    for c in range(N_CORES):
        tg, ng = divmod(c, NGROUPS)
        full[
            tg * T_SHARD:(tg + 1) * T_SHARD, ng * N_SHARD:(ng + 1) * N_SHARD
        ] = res.results[c]["out"]
    return full.reshape(B, S, DOUT).astype(np.float32)
